# revision 69
# baseline (speedup 1.0000x reference)
"""Trainium2 Bass kernel: dense transformer block (LN1-attn-LN2-FFN, causal, 16 heads).

Sharding (8 NeuronCores, SPMD one graph):
  - core j: token-parallel for LN/FFN/residual: owns tokens [256j, 256(j+1))
    of BOTH batches (so per-batch collectives stay 8-way uniform).
  - attention head-parallel: core j computes head pair {2j, 2j+1} for both
    batches over the full 2048-token sequence.
  - comm: per-batch AllGather of feature-major LN1 output (batch-1 AG
    overlaps batch-0 QKV), per-batch AllToAll of normalized attention output
    (batch-0's A2A + wo projection + LN2 hide under batch-1 attention; A2A
    input staged per quarter-round).
  - attention is kt-pair-major: both heads' score matmuls pack the PE array
    via row tiles (K=64 each); exp on ACT software-pipelined per kt; causal
    masks multiplied on DVE; PV runs as fp8 DoubleRow matmuls over kt pairs
    (vtok holds 64*v in fp8 with ones-columns = 64 so the softmax
    normalization ratio cancels the scale); denominator replicated across
    PSUM partitions 64:127, normalized via approx-reciprocal + DVE multiply.
  - batch-1 QKV chunks are interleaved into batch-0's attention rounds
    (shared PSUM score slots) to fill PE while ACT drains exp.
  - wo projection in fp8 DoubleRow, split by token half.
  - FFN bf16 (fp8 fails the accuracy gate: per-element fp8 error does not
    average out in dot products); matmuls f32-accumulate; residual f32.
  - LN gains/biases, 1/sqrt(dk), and bv are folded into weights host-side.
"""

import numpy as np
import ml_dtypes

import concourse.bass as bass
import concourse.tile as tile
from concourse import bacc, mybir
from concourse.bass_utils import run_bass_kernel_spmd

F32 = mybir.dt.float32
BF16 = mybir.dt.bfloat16
AF = mybir.ActivationFunctionType

D = 1024
DFF = 4096
B = 2
S = 2048
NCORES = 8
GRP = 4
TOK = 512
EPS = 1e-5

AGH = 128 * 8 * 256      # elems contributed per core per AG half (fp8)
A2A_N = NCORES * 128 * 256   # per-batch AllToAll payload (fp8)


def build_nc():
    nc = bacc.Bacc("TRN2", target_bir_lowering=False, debug=False,
                   num_devices=NCORES)

    x_own = nc.dram_tensor("x_own", [TOK, D], F32, kind="ExternalInput").ap()
    wq = nc.dram_tensor("wq", [128, 8, 128], mybir.dt.float8e4,
                        kind="ExternalInput").ap()
    wk = nc.dram_tensor("wk", [128, 8, 128], mybir.dt.float8e4,
                        kind="ExternalInput").ap()
    wv = nc.dram_tensor("wv", [128, 8, 128], mybir.dt.float8e4,
                        kind="ExternalInput").ap()
    bq = nc.dram_tensor("bq", [1, 128], F32, kind="ExternalInput").ap()
    bk = nc.dram_tensor("bk", [1, 128], F32, kind="ExternalInput").ap()
    wo = nc.dram_tensor("wo", [128, 8, D], mybir.dt.float8e4,
                        kind="ExternalInput").ap()
    bo = nc.dram_tensor("bo", [D], F32, kind="ExternalInput").ap()
    w1 = nc.dram_tensor("w1", [128, 32, 8, 128], BF16, kind="ExternalInput").ap()
    b1 = nc.dram_tensor("b1", [DFF], F32, kind="ExternalInput").ap()
    w2 = nc.dram_tensor("w2", [128, 8, 4, 8, 128], BF16, kind="ExternalInput").ap()
    b2 = nc.dram_tensor("b2", [D], F32, kind="ExternalInput").ap()
    me = nc.dram_tensor("me", [128, 256], mybir.dt.float8e4,
                        kind="ExternalInput").ap()
    mo = nc.dram_tensor("mo", [128, 256], mybir.dt.float8e4,
                        kind="ExternalInput").ap()
    id128 = nc.dram_tensor("id128", [128, 128], BF16, kind="ExternalInput").ap()
    out = nc.dram_tensor("out", [TOK, D], F32, kind="ExternalOutput").ap()

    rg = [list(range(NCORES))]

    with tile.TileContext(nc) as tc:
        with (
            tc.tile_pool(name="persist", bufs=1) as pp,
            tc.tile_pool(name="stage", bufs=4) as stg,
            tc.tile_pool(name="stats", bufs=4) as stp,
            tc.tile_pool(name="dram", bufs=1, space="DRAM") as dp,
        ):
            # ---- persistent SBUF (weights on ACT queue; x on SP queue) ----
            eps_sb = pp.tile([128, 1], F32, name="eps_sb")
            nc.vector.memset(eps_sb, EPS)

            id_sb = pp.tile([128, 128], BF16, name="id_sb")
            nc.scalar.dma_start(out=id_sb, in_=id128)
            me_sb = pp.tile([128, 256], mybir.dt.float8e4, name="me_sb")
            mo_sb = pp.tile([128, 256], mybir.dt.float8e4, name="mo_sb")
            bo_sb = pp.tile([128, 8], F32, name="bo_sb")
            b1_sb = pp.tile([128, 32], F32, name="b1_sb")
            b2_sb = pp.tile([128, 8], F32, name="b2_sb")

            x_tok = pp.tile([128, 4, D], F32, name="x_tok")
            x4 = x_own.rearrange("(t p) d -> t p d", p=128)
            for t in range(4):
                nc.sync.dma_start(out=x_tok[:, t, :], in_=x4[t])

            wq_sb = pp.tile([128, 8, 128], mybir.dt.float8e4, name="wq_sb")
            nc.scalar.dma_start(out=wq_sb, in_=wq)
            wk_sb = pp.tile([128, 8, 128], mybir.dt.float8e4, name="wk_sb")
            nc.scalar.dma_start(out=wk_sb, in_=wk)
            wv_sb = pp.tile([128, 8, 128], mybir.dt.float8e4, name="wv_sb")
            nc.scalar.dma_start(out=wv_sb, in_=wv)
            bq_sb = pp.tile([128, 1], F32, name="bq_sb")
            bk_sb = pp.tile([128, 1], F32, name="bk_sb")

            qT = pp.tile([128, B, S], BF16, name="qT")
            kT = pp.tile([128, B, S], BF16, name="kT")
            # vtok holds 64*v in fp8 (wv is pre-scaled x64 host-side); the
            # ones-columns are 64.0 too, so the normalization ratio cancels
            # the scale exactly.
            vtok = pp.tile([128, 32, 2, 128], mybir.dt.float8e4, name="vtok")
            nc.vector.memset(vtok[:, :, :, 64:128], 64.0)
            attnT = pp.tile([128, B, S], BF16, name="attnT")
            attn8 = pp.tile([128, B, S], mybir.dt.float8e4, name="attn8")
            lnT = pp.tile([128, 8, TOK], BF16, name="lnT")
            lnT1 = pp.tile([128, 8, TOK], mybir.dt.float8e4, name="lnT1")

            # ---- DRAM ----
            ag_in = [dp.tile([AGH], mybir.dt.float8e4, name=f"ag_in{h}")
                     for h in range(2)]
            ag_out = [dp.tile([NCORES * AGH], mybir.dt.float8e4,
                              name=f"ag_out{h}", addr_space="Shared")
                      for h in range(2)]
            a2a_in = [dp.tile([A2A_N], mybir.dt.float8e4, name=f"a2a_in{b}")
                      for b in range(B)]
            a2a_out = [dp.tile([A2A_N], mybir.dt.float8e4, name=f"a2a_out{b}")
                       for b in range(B)]

            # ---- LN helper (token-major stats; bf16 out) ----
            def layernorm_chunk(src, t, xout):
                # src [128, 4, 1024] f32; writes xout [128, 1024] bf16
                xin = src[:, t, :]
                xg_ = xin.rearrange("p (g d) -> p g d", g=2)
                st_ = stp.tile([128, 2, 6], F32, name="st_")
                for gs in range(2):
                    nc.vector.bn_stats(out=st_[:, gs, :], in_=xg_[:, gs, :])
                mv = stp.tile([128, 2], F32, name="mv")
                nc.vector.bn_aggr(out=mv, in_=st_)
                rstd = stp.tile([128, 1], F32, name="rstd")
                nc.scalar.activation(out=rstd, in_=mv[:, 1:2], func=AF.Sqrt,
                                     bias=eps_sb, scale=1.0)
                nc.vector.reciprocal(out=rstd, in_=rstd)
                nc.vector.tensor_scalar(out=xout, in0=xin, scalar1=mv[:, 0:1],
                                        scalar2=rstd,
                                        op0=mybir.AluOpType.subtract,
                                        op1=mybir.AluOpType.mult)

            # xg: [128 feat, src, batch, kc, 256 tok] fp8 (persists through
            # attention so batch-1 QKV can interleave with batch-0 rounds)
            xg = pp.tile([128, NCORES, 2, 8, 256], mybir.dt.float8e4,
                         name="xg")

            with tc.tile_pool(name="psT1", bufs=2, space="PSUM") as psT1:
                def ln_transpose(src, t):
                    # LN of token chunk t -> feature-major lnT1 (fp8 cast in
                    # the PSUM->SBUF copy)
                    xln = stg.tile([128, D], BF16, name="xln")
                    layernorm_chunk(src, t, xln)
                    for fb in range(8):
                        pt = psT1.tile([128, 128], BF16, name="pt")
                        nc.tensor.transpose(pt, xln[:, fb * 128:(fb + 1) * 128],
                                            id_sb)
                        nc.vector.tensor_copy(
                            out=lnT1[:, fb, t * 128:(t + 1) * 128], in_=pt)

                # ---- LN1 + staged AllGather (2 token-halves = batches) ----
                for h in range(2):
                    for tt in range(2):
                        ln_transpose(x_tok, 2 * h + tt)
                    nc.sync.dma_start(
                        out=ag_in[h].rearrange("(p k t) -> p k t", p=128, k=8),
                        in_=lnT1[:, :, h * 256:(h + 1) * 256])
                    nc.gpsimd.collective_compute(
                        "AllGather", mybir.AluOpType.bypass, replica_groups=rg,
                        ins=[ag_in[h].opt()], outs=[ag_out[h].opt()])
                    if h == 0:
                        nc.gpsimd.dma_start(out=bq_sb,
                                            in_=bq.rearrange("b p -> p b"))
                        nc.gpsimd.dma_start(out=bk_sb,
                                            in_=bk.rearrange("b p -> p b"))
                        nc.gpsimd.dma_start(out=me_sb, in_=me)
                        nc.gpsimd.dma_start(out=mo_sb, in_=mo)
                        nc.gpsimd.dma_start(
                            out=bo_sb,
                            in_=bo.rearrange("(k p) -> p k", p=128))
                        nc.gpsimd.dma_start(
                            out=b1_sb,
                            in_=b1.rearrange("(k p) -> p k", p=128))
                        nc.gpsimd.dma_start(
                            out=b2_sb,
                            in_=b2.rearrange("(k p) -> p k", p=128))

                for h in range(2):
                    ago = ag_out[h].rearrange("(r p k t) -> r p k t",
                                              r=NCORES, p=128, k=8)
                    for r in range(NCORES):
                        eng = [nc.sync, nc.scalar][r % 2]
                        eng.dma_start(out=xg[:, r, h], in_=ago[r])

            # ---- attention: kt-major, 2-head row-packed scores ----
            with tc.tile_pool(name="ffp", bufs=1) as ffp:
                # preload the Exp table while ACT is idle (after LN1 Sqrt)
                wrm = ffp.tile([128, 1], F32, name="wrm")
                nc.scalar.activation(out=wrm, in_=eps_sb, func=AF.Exp)
                af_sb = ffp.tile([128, 8, TOK], mybir.dt.float8e4,
                                 name="af_sb")
                wo_sb = ffp.tile([128, 8, D], mybir.dt.float8e4,
                                 name="wo_sb")
                nc.scalar.dma_start(out=wo_sb, in_=wo)
                h1T = ffp.tile([128, 32, TOK], BF16, name="h1T")

                with (
                    tc.tile_pool(name="ptp", bufs=4) as ptp,
                    tc.tile_pool(name="rp", bufs=6) as rp,
                    tc.tile_pool(name="psS", bufs=2, space="PSUM") as psS,
                    tc.tile_pool(name="psO", bufs=3, space="PSUM") as psO,
                ):
                    DR = mybir.MatmulPerfMode.DoubleRow

                    def wo_half(half, accpool, ptpool, acctag, pttag,
                                tbufs=None):
                        # wo projection + residual for token half `half`
                        # (256 tokens); half 0 runs inside batch-1 attention,
                        # hidden under the A2A(b=1) wait.
                        for fb in range(8):
                            accw = accpool.tile([128, 256], F32, name="accw",
                                                tag=acctag, bufs=tbufs)
                            for c in range(4):
                                nc.tensor.matmul(
                                    accw,
                                    lhsT=wo_sb[:, 2 * c:2 * c + 2,
                                               fb * 128:(fb + 1) * 128],
                                    rhs=af_sb[:, 2 * c:2 * c + 2,
                                              half * 256:(half + 1) * 256],
                                    start=(c == 0), stop=(c == 3),
                                    perf_mode=DR)
                            yT = stg.tile([128, 256], BF16, name="yT")
                            nc.vector.tensor_scalar(
                                out=yT, in0=accw, scalar1=1.0 / 64.0,
                                scalar2=bo_sb[:, fb:fb + 1],
                                op0=mybir.AluOpType.mult,
                                op1=mybir.AluOpType.add)
                            for tt in range(2):
                                t = 2 * half + tt
                                ptw = ptpool.tile([128, 128], BF16,
                                                  name="ptw", tag=pttag,
                                                  bufs=tbufs)
                                nc.tensor.transpose(
                                    ptw, yT[:, tt * 128:(tt + 1) * 128],
                                    id_sb)
                                nc.vector.tensor_add(
                                    out=x_tok[:, t, fb * 128:(fb + 1) * 128],
                                    in0=x_tok[:, t, fb * 128:(fb + 1) * 128],
                                    in1=ptw)

                    def ln2_chunk(t, ptpool, pttag, tbufs=None):
                        xln = stg.tile([128, D], BF16, name="xln")
                        layernorm_chunk(x_tok, t, xln)
                        for fb in range(8):
                            ptl = ptpool.tile([128, 128], BF16, name="ptl",
                                              tag=pttag, bufs=tbufs)
                            nc.tensor.transpose(
                                ptl, xln[:, fb * 128:(fb + 1) * 128], id_sb)
                            nc.vector.tensor_copy(
                                out=lnT[:, fb, t * 128:(t + 1) * 128],
                                in_=ptl)

                    def qkv_chunk(h, r):
                        # QKV for tokens [256r, 256(r+1)) of batch h.  PSUM
                        # accumulators share the attention score slots (tag
                        # "sc") so batch-1 chunks can interleave with batch-0
                        # attention rounds without extra PSUM banks.
                        c0 = r * 256
                        for dst, wsb, bsb in ((qT, wq_sb, bq_sb),
                                              (kT, wk_sb, bk_sb)):
                            acc = psS.tile([128, 256], F32, name="acc",
                                           tag="sc")
                            for kc in range(8):
                                nc.tensor.matmul(acc, lhsT=wsb[:, kc, :],
                                                 rhs=xg[:, r, h, kc, :],
                                                 start=(kc == 0),
                                                 stop=(kc == 7))
                            nc.vector.tensor_scalar(
                                out=dst[:, h, c0:c0 + 256], in0=acc,
                                scalar1=1.0 / 64.0,
                                scalar2=bsb,
                                op0=mybir.AluOpType.mult,
                                op1=mybir.AluOpType.add)
                        for tc_ in range(2):
                            accv = psS.tile([128, 128], F32, name="accv",
                                            tag="sc")
                            for kc in range(8):
                                nc.tensor.matmul(
                                    accv,
                                    lhsT=xg[:, r, h, kc,
                                            tc_ * 128:tc_ * 128 + 128],
                                    rhs=wv_sb[:, kc, :],
                                    start=(kc == 0), stop=(kc == 7))
                            ch = h * 16 + 2 * r + tc_
                            nc.vector.tensor_copy(
                                out=vtok[:, ch, :, 0:64],
                                in_=accv.rearrange("p (hd c) -> p hd c",
                                                   hd=2))

                    def emit_pv(b, q2lo_h, q2hi, Ot, m, q2lo_p, pt2):
                        # PV over the kt pair (2m, 2m+1) as one fp8 DoubleRow
                        # matmul (K=256 virtual): lhsT [128,2,128], rhs
                        # [128,2,256] interleaved k-halves.
                        for q2 in range(q2lo_p, q2hi):
                            qr = (q2 - q2lo_p) * 256
                            for hh in range(2):
                                # start=True clears has_written bits for the
                                # WHOLE bank; two q2 share one bank, so only
                                # the bank-first q2 may clear. The sibling's
                                # first write lands on cleared bits and
                                # overwrites correctly.
                                nc.tensor.matmul(
                                    Ot[hh][:, q2 - q2lo_h, :],
                                    lhsT=vtok[:, b * 16 + 2 * m:
                                              b * 16 + 2 * m + 2, hh, :],
                                    rhs=pt2[:, :, hh, qr:qr + 256],
                                    start=(m == 0
                                           and (q2 - q2lo_h) % 2 == 0),
                                    stop=(m == q2),
                                    perf_mode=DR,
                                    skip_group_check=True)

                    # dense QKV for batch 0; batch-1 chunks are spread into
                    # batch-0's attention rounds (PE fills exp-wait gaps)
                    for r in range(NCORES):
                        qkv_chunk(0, r)

                    # quarter-rounds: (b, qq) covers q2 pair (2qq, 2qq+1)
                    for b in range(B):
                        fills = list(range(NCORES)) if b == 0 else []
                        for qq in range(4):
                            q2lo_h, q2hi = 2 * qq, 2 * qq + 2
                            npair = 2 * qq + 2
                            # O (per head): [128, 2 q2, 256]; rows 64:127 get
                            # the denominator replicated via vtok's ones cols
                            Ot = [psO.tile([128, 2, 256], F32, name="O")
                                  for _ in range(2)]
                            pend = None
                            for m in range(npair):
                                q2lo_p = max(q2lo_h, m)
                                ncols = (q2hi - q2lo_p) * 256
                                # probs for kt pair: [p, j, hh, q] fp8
                                pt2 = ptp.tile([128, 2, 2, 512],
                                               mybir.dt.float8e4, name="pt2")
                                for j in range(2):
                                    kt = 2 * m + j
                                    sc = psS.tile([128, 2, 512], F32,
                                                  name="sc")
                                    for hh in range(2):
                                        hp = hh * 64
                                        nc.tensor.matmul(
                                            sc[:, hh, :ncols],
                                            lhsT=kT[hp:hp + 64, b,
                                                    kt * 128:kt * 128 + 128],
                                            rhs=qT[hp:hp + 64, b,
                                                   q2lo_p * 256:
                                                   q2lo_p * 256 + ncols],
                                            start=True, stop=True)
                                    nc.scalar.activation(
                                        out=pt2[:, j, :, :ncols],
                                        in_=sc[:, :, :ncols], func=AF.Exp)
                                    # causal mask on the diagonal 256-block
                                    if m >= q2lo_h:
                                        msk = me_sb if j == 0 else mo_sb
                                        for hh in range(2):
                                            nc.vector.tensor_mul(
                                                out=pt2[:, j, hh, 0:256],
                                                in0=pt2[:, j, hh, 0:256],
                                                in1=msk)
                                # software pipeline: PV of the previous pair
                                # issues after this pair's score matmuls
                                if pend is not None:
                                    emit_pv(b, q2lo_h, q2hi, Ot, *pend)
                                pend = (m, q2lo_p, pt2)
                            emit_pv(b, q2lo_h, q2hi, Ot, *pend)

                            # normalize: denominator rows are broadcast by the
                            # matmul; copy to SBUF (approx-recip can't read
                            # PSUM), reciprocal, fused multiply per head
                            for hh in range(2):
                                hp = hh * 64
                                for q2 in range(q2lo_h, q2hi):
                                    O2 = Ot[hh][:, q2 - q2lo_h, :]
                                    den = rp.tile([64, 256], F32, name="den")
                                    nc.vector.tensor_copy(out=den,
                                                          in_=O2[64:128, :])
                                    rc = rp.tile([64, 256], F32, name="rc")
                                    nc.vector.reciprocal_approx_fast(
                                        out=rc, in_=den)
                                    nc.vector.tensor_mul(
                                        out=attnT[hp:hp + 64, b,
                                                  q2 * 256:q2 * 256 + 256],
                                        in0=O2[0:64, :], in1=rc)
                            # fp8-cast this round's block (cast must be a
                            # tensor_copy for neuronxcc); round qq maps
                            # exactly to A2A dest slices {2qq, 2qq+1}, so
                            # stage it immediately
                            nc.vector.tensor_copy(
                                out=attn8[:, b, q2lo_h * 256:q2hi * 256],
                                in_=attnT[:, b, q2lo_h * 256:q2hi * 256])
                            nc.sync.dma_start(
                                out=a2a_in[b].rearrange(
                                    "(s p t) -> p s t", s=8,
                                    p=128)[:, 2 * qq:2 * qq + 2],
                                in_=attn8[:, b, q2lo_h * 256:q2hi * 256]
                                .rearrange("p (s t) -> p s t", s=2))

                            # spread batch-1 QKV into batch-0's rounds: the
                            # PE picks these up while ACT drains this round's
                            # exp backlog
                            for _ in range(2):
                                if fills:
                                    qkv_chunk(1, fills.pop(0))

                        # AllToAll per batch (b=0's collective hides under
                        # the b=1 compute)
                        nc.gpsimd.collective_compute(
                            "AllToAll", mybir.AluOpType.bypass,
                            replica_groups=rg,
                            ins=[a2a_in[b].opt()], outs=[a2a_out[b].opt()])
                        _af3 = a2a_out[b].rearrange("(i p t) -> i p t",
                                                    i=8, p=128)
                        for i in range(8):
                            [nc.gpsimd, nc.scalar][i % 2].dma_start(
                                out=af_sb[:, i, b * 256:(b + 1) * 256],
                                in_=_af3[i])
                        if b == 0:
                            # batch-0's wo + residual + LN2 run inside
                            # batch-1 attention / the A2A(b=1) wait, through
                            # a dedicated 1-bank PSUM slot so they never
                            # gate the attention Ot slots
                            wo_half(0, psO, psO, "wo", "wo", tbufs=1)
                            ln2_chunk(0, psO, "wo", tbufs=1)
                            ln2_chunk(1, psO, "wo", tbufs=1)

                # ---- wo projection + residual; LN2; FFN ----
                with (
                    tc.tile_pool(name="psW", bufs=3, space="PSUM") as psW,
                    tc.tile_pool(name="psT2", bufs=2, space="PSUM") as psT2,
                    tc.tile_pool(name="w2p", bufs=6) as w2p,
                ):
                    def resid_add(ybf, fb):
                        # ybf [128 feat, 512 tok] bf16 -> x_tok += y^T
                        for t in range(4):
                            pt = psT2.tile([128, 128], BF16, name="pt")
                            nc.tensor.transpose(
                                pt, ybf[:, t * 128:(t + 1) * 128], id_sb)
                            nc.vector.tensor_add(
                                out=x_tok[:, t, fb * 128:(fb + 1) * 128],
                                in0=x_tok[:, t, fb * 128:(fb + 1) * 128],
                                in1=pt)

                    # wo + residual + LN2 for the batch-1 token half (the
                    # batch-0 half ran inside the attention region)
                    wo_half(1, psW, psT2, "acc", "pt")
                    ln2_chunk(2, psT2, "pt")
                    ln2_chunk(3, psT2, "pt")

                    # ---- FFN1 ----
                    with tc.tile_pool(name="w1p", bufs=6) as w1p:
                        for hbk in range(32):
                            w1t = w1p.tile([128, 8, 128], BF16, name="w1t")
                            nc.sync.dma_start(out=w1t, in_=w1[:, hbk])
                            acc = psW.tile([128, TOK], F32, name="acc")
                            for kc in range(8):
                                nc.tensor.matmul(acc, lhsT=w1t[:, kc, :],
                                                 rhs=lnT[:, kc, :],
                                                 start=(kc == 0),
                                                 stop=(kc == 7))
                            nc.scalar.activation(out=h1T[:, hbk, :], in_=acc,
                                                 func=AF.Gelu,
                                                 bias=b1_sb[:, hbk:hbk + 1],
                                                 scale=1.0)

                    # ---- FFN2 + residual + streamed output ----
                    for fb in range(8):
                        acc = psW.tile([128, TOK], F32, name="acc")
                        for hg in range(4):
                            w2t = w2p.tile([128, 8, 128], BF16, name="w2t")
                            nc.sync.dma_start(out=w2t, in_=w2[:, fb, hg])
                            for kc in range(8):
                                nc.tensor.matmul(acc, lhsT=w2t[:, kc, :],
                                                 rhs=h1T[:, hg * 8 + kc, :],
                                                 start=(hg == 0 and kc == 0),
                                                 stop=(hg == 3 and kc == 7))
                        y2T = stg.tile([128, TOK], BF16, name="y2T")
                        nc.vector.tensor_scalar_add(out=y2T, in0=acc,
                                                    scalar1=b2_sb[:, fb:fb + 1])
                        resid_add(y2T, fb)
                        ov = out.rearrange("(t p) d -> p t d", p=128)
                        for t in range(4):
                            [nc.sync, nc.scalar][t % 2].dma_start(
                                out=ov[:, t, fb * 128:(fb + 1) * 128],
                                in_=x_tok[:, t, fb * 128:(fb + 1) * 128])

    nc.compile()
    return nc


_NC_CACHE = {}


def _get_nc():
    if "nc" not in _NC_CACHE:
        _NC_CACHE["nc"] = build_nc()
    return _NC_CACHE["nc"]


def _prep_in_maps(x, ln1_g, ln1_b, wq, bq, wk, bk, wv, bv, wo, bo,
                  ln2_g, ln2_b, w1, b1, w2, b2):
    bf16 = ml_dtypes.bfloat16
    f32 = np.float32
    x = np.asarray(x, f32)
    DK = 64
    sc = 1.0 / np.sqrt(DK)
    ln1_g = np.asarray(ln1_g, f32)
    ln1_b = np.asarray(ln1_b, f32)
    ln2_g = np.asarray(ln2_g, f32)
    ln2_b = np.asarray(ln2_b, f32)
    wq = np.asarray(wq, f32)
    wk = np.asarray(wk, f32)
    wv = np.asarray(wv, f32)
    wo_np = np.asarray(wo, f32)
    w1 = np.asarray(w1, f32)
    w2 = np.asarray(w2, f32)

    f8 = ml_dtypes.float8_e4m3
    wq_f = (ln1_g[:, None] * wq * sc * 64.0).astype(f8)
    bq_f = ((ln1_b @ wq + np.asarray(bq, f32)) * sc).astype(f32)
    wk_f = (ln1_g[:, None] * wk * 64.0).astype(f8)
    bk_f = (ln1_b @ wk + np.asarray(bk, f32)).astype(f32)
    wv_f = (ln1_g[:, None] * wv * 64.0).astype(f8)
    bv_f = (ln1_b @ wv + np.asarray(bv, f32)).astype(f32)
    bo_f = (np.asarray(bo, f32) + bv_f @ wo_np).astype(f32)
    wo_f = (wo_np * 64.0).astype(ml_dtypes.float8_e4m3)
    w1_f = (ln2_g[:, None] * w1).astype(bf16)
    b1_f = (ln2_b @ w1 + np.asarray(b1, f32)).astype(f32)
    w2_f = w2.astype(bf16)
    b2_f = np.asarray(b2, f32)

    tri = np.triu(np.ones((128, 128), f32))
    me_np = np.concatenate([tri, np.ones((128, 128), f32)], 1).astype(f8)
    mo_np = np.concatenate([np.zeros((128, 128), f32), tri], 1).astype(f8)
    id_np = np.eye(128, dtype=f32).astype(bf16)

    def pmaj(a):   # [1024, ...] -> [128, 8, ...] partition-major
        return np.ascontiguousarray(
            a.reshape(8, 128, *a.shape[1:]).transpose(
                1, 0, *range(2, a.ndim + 1)))

    w1_pm = np.ascontiguousarray(
        w1_f.reshape(8, 128, 32, 128).transpose(1, 2, 0, 3))
    w2_pm = np.ascontiguousarray(
        w2_f.reshape(4, 8, 128, 8, 128).transpose(2, 3, 0, 1, 4))
    wo_pm = pmaj(wo_f)

    in_maps = []
    for core in range(NCORES):
        # core j: owns tokens [256j, 256(j+1)) of BOTH batches; computes
        # heads {2j, 2j+1} for both batches.
        hs = slice(core * 128, (core + 1) * 128)
        in_maps.append({
            "x_own": np.ascontiguousarray(np.concatenate(
                [x[b, core * 256:(core + 1) * 256, :] for b in range(B)],
                axis=0)),
            "wq": pmaj(wq_f[:, hs]),
            "wk": pmaj(wk_f[:, hs]),
            "wv": pmaj(wv_f[:, hs]),
            "bq": np.ascontiguousarray(bq_f[None, hs]),
            "bk": np.ascontiguousarray(bk_f[None, hs]),
            "wo": wo_pm, "bo": bo_f,
            "w1": w1_pm, "b1": b1_f,
            "w2": w2_pm, "b2": b2_f,
            "me": me_np, "mo": mo_np, "id128": id_np,
        })
    return in_maps


def kernel(**inputs):
    nc = _get_nc()
    in_maps = _prep_in_maps(**inputs)
    res = run_bass_kernel_spmd(nc, in_maps, core_ids=list(range(NCORES)))
    full = np.empty((B, S, D), np.float32)
    for core in range(NCORES):
        o = res.results[core]["out"]
        for b in range(B):
            full[b, core * 256:(core + 1) * 256, :] = \
                o[b * 256:(b + 1) * 256, :]
    return full



# revision 72
# speedup vs baseline: 1.0375x; 1.0375x over previous
"""Trainium2 Bass kernel: dense transformer block (LN1-attn-LN2-FFN, causal, 16 heads).

Sharding (8 NeuronCores, SPMD one graph):
  - core j: token-parallel for LN/FFN/residual: owns tokens [256j, 256(j+1))
    of BOTH batches (so per-batch collectives stay 8-way uniform).
  - attention head-parallel: core j computes head pair {2j, 2j+1} for both
    batches over the full 2048-token sequence.
  - comm: per-batch AllGather of feature-major LN1 output (batch-1 AG
    overlaps batch-0 QKV), per-batch AllToAll of normalized attention output
    (batch-0's A2A + wo projection + LN2 hide under batch-1 attention; A2A
    input staged per quarter-round).
  - attention is kt-pair-major: both heads' score matmuls pack the PE array
    via row tiles (K=64 each); exp on ACT software-pipelined per kt; causal
    masks multiplied on DVE; PV runs as fp8 DoubleRow matmuls over kt pairs
    (vtok holds 64*v in fp8 with ones-columns = 64 so the softmax
    normalization ratio cancels the scale); denominator replicated across
    PSUM partitions 64:127, normalized via approx-reciprocal + DVE multiply.
  - batch-1 QKV chunks are interleaved into batch-0's attention rounds
    (shared PSUM score slots) to fill PE while ACT drains exp.
  - wo projection in fp8 DoubleRow, split by token half.
  - FFN bf16 (fp8 fails the accuracy gate: per-element fp8 error does not
    average out in dot products); matmuls f32-accumulate; residual f32.
  - LN gains/biases, 1/sqrt(dk), and bv are folded into weights host-side.
"""

import numpy as np
import ml_dtypes

import concourse.bass as bass
import concourse.tile as tile
from concourse import bacc, mybir
from concourse.bass_utils import run_bass_kernel_spmd

F32 = mybir.dt.float32
BF16 = mybir.dt.bfloat16
AF = mybir.ActivationFunctionType

D = 1024
DFF = 4096
B = 2
S = 2048
NCORES = 8
GRP = 4
TOK = 512
EPS = 1e-5

AGH = 128 * 8 * 256      # elems contributed per core per AG half (fp8)
A2A_N = NCORES * 128 * 256   # per-batch AllToAll payload (fp8)


def build_nc():
    nc = bacc.Bacc("TRN2", target_bir_lowering=False, debug=False,
                   num_devices=NCORES)

    x_own = nc.dram_tensor("x_own", [TOK, D], F32, kind="ExternalInput").ap()
    wq = nc.dram_tensor("wq", [128, 8, 128], mybir.dt.float8e4,
                        kind="ExternalInput").ap()
    wk = nc.dram_tensor("wk", [128, 8, 128], mybir.dt.float8e4,
                        kind="ExternalInput").ap()
    wv = nc.dram_tensor("wv", [128, 8, 128], mybir.dt.float8e4,
                        kind="ExternalInput").ap()
    bq = nc.dram_tensor("bq", [1, 128], F32, kind="ExternalInput").ap()
    bk = nc.dram_tensor("bk", [1, 128], F32, kind="ExternalInput").ap()
    wo = nc.dram_tensor("wo", [128, 8, D], mybir.dt.float8e4,
                        kind="ExternalInput").ap()
    bo = nc.dram_tensor("bo", [D], F32, kind="ExternalInput").ap()
    w1 = nc.dram_tensor("w1", [128, 32, 8, 128], BF16, kind="ExternalInput").ap()
    b1 = nc.dram_tensor("b1", [DFF], F32, kind="ExternalInput").ap()
    w2 = nc.dram_tensor("w2", [128, 8, 4, 8, 128], BF16, kind="ExternalInput").ap()
    b2 = nc.dram_tensor("b2", [D], F32, kind="ExternalInput").ap()
    me = nc.dram_tensor("me", [128, 256], mybir.dt.float8e4,
                        kind="ExternalInput").ap()
    mo = nc.dram_tensor("mo", [128, 256], mybir.dt.float8e4,
                        kind="ExternalInput").ap()
    id128 = nc.dram_tensor("id128", [128, 128], BF16, kind="ExternalInput").ap()
    out = nc.dram_tensor("out", [TOK, D], F32, kind="ExternalOutput").ap()

    rg = [list(range(NCORES))]

    with tile.TileContext(nc) as tc:
        with (
            tc.tile_pool(name="persist", bufs=1) as pp,
            tc.tile_pool(name="stage", bufs=4) as stg,
            tc.tile_pool(name="stats", bufs=4) as stp,
            tc.tile_pool(name="dram", bufs=1, space="DRAM") as dp,
        ):
            # ---- persistent SBUF (weights on ACT queue; x on SP queue) ----
            eps_sb = pp.tile([128, 1], F32, name="eps_sb")
            nc.vector.memset(eps_sb, EPS)

            id_sb = pp.tile([128, 128], BF16, name="id_sb")
            nc.scalar.dma_start(out=id_sb, in_=id128)
            me_sb = pp.tile([128, 256], mybir.dt.float8e4, name="me_sb")
            mo_sb = pp.tile([128, 256], mybir.dt.float8e4, name="mo_sb")
            bo_sb = pp.tile([128, 8], F32, name="bo_sb")
            b1_sb = pp.tile([128, 32], F32, name="b1_sb")
            b2_sb = pp.tile([128, 8], F32, name="b2_sb")

            x_tok = pp.tile([128, 4, D], F32, name="x_tok")
            x4 = x_own.rearrange("(t p) d -> t p d", p=128)
            for t in range(4):
                nc.sync.dma_start(out=x_tok[:, t, :], in_=x4[t])

            wq_sb = pp.tile([128, 8, 128], mybir.dt.float8e4, name="wq_sb")
            nc.scalar.dma_start(out=wq_sb, in_=wq)
            wk_sb = pp.tile([128, 8, 128], mybir.dt.float8e4, name="wk_sb")
            nc.scalar.dma_start(out=wk_sb, in_=wk)
            wv_sb = pp.tile([128, 8, 128], mybir.dt.float8e4, name="wv_sb")
            nc.scalar.dma_start(out=wv_sb, in_=wv)
            bq_sb = pp.tile([128, 1], F32, name="bq_sb")
            bk_sb = pp.tile([128, 1], F32, name="bk_sb")

            qT = pp.tile([128, B, S], BF16, name="qT")
            kT = pp.tile([128, B, S], BF16, name="kT")
            # vtok holds 64*v in fp8 (wv is pre-scaled x64 host-side); the
            # ones-columns are 64.0 too, so the normalization ratio cancels
            # the scale exactly.
            vtok = pp.tile([128, 32, 2, 128], mybir.dt.float8e4, name="vtok")
            nc.vector.memset(vtok[:, :, :, 64:128], 64.0)
            attnT = pp.tile([128, B, S], BF16, name="attnT")
            attn8 = pp.tile([128, B, S], mybir.dt.float8e4, name="attn8")
            lnT = pp.tile([128, 8, TOK], BF16, name="lnT")
            lnT1 = pp.tile([128, 8, TOK], mybir.dt.float8e4, name="lnT1")

            # ---- DRAM ----
            ag_in = [dp.tile([AGH], mybir.dt.float8e4, name=f"ag_in{h}")
                     for h in range(2)]
            ag_out = [dp.tile([NCORES * AGH], mybir.dt.float8e4,
                              name=f"ag_out{h}", addr_space="Shared")
                      for h in range(2)]
            a2a_in = [dp.tile([A2A_N], mybir.dt.float8e4, name=f"a2a_in{b}")
                      for b in range(B)]
            a2a_out = [dp.tile([A2A_N], mybir.dt.float8e4, name=f"a2a_out{b}")
                       for b in range(B)]

            # ---- LN helper (token-major stats; bf16 out) ----
            def layernorm_chunk(src, t, xout):
                # src [128, 4, 1024] f32; writes xout [128, 1024] bf16
                xin = src[:, t, :]
                xg_ = xin.rearrange("p (g d) -> p g d", g=2)
                st_ = stp.tile([128, 2, 6], F32, name="st_")
                for gs in range(2):
                    nc.vector.bn_stats(out=st_[:, gs, :], in_=xg_[:, gs, :])
                mv = stp.tile([128, 2], F32, name="mv")
                nc.vector.bn_aggr(out=mv, in_=st_)
                rstd = stp.tile([128, 1], F32, name="rstd")
                nc.scalar.activation(out=rstd, in_=mv[:, 1:2], func=AF.Sqrt,
                                     bias=eps_sb, scale=1.0)
                nc.vector.reciprocal(out=rstd, in_=rstd)
                nc.vector.tensor_scalar(out=xout, in0=xin, scalar1=mv[:, 0:1],
                                        scalar2=rstd,
                                        op0=mybir.AluOpType.subtract,
                                        op1=mybir.AluOpType.mult)

            # xg: [128 feat, src, batch, kc, 256 tok] fp8 (persists through
            # attention so batch-1 QKV can interleave with batch-0 rounds)
            xg = pp.tile([128, NCORES, 2, 8, 256], mybir.dt.float8e4,
                         name="xg")

            with tc.tile_pool(name="psT1", bufs=2, space="PSUM") as psT1:
                def ln_transpose(src, t):
                    # LN of token chunk t -> feature-major lnT1 (fp8 cast in
                    # the PSUM->SBUF copy)
                    xln = stg.tile([128, D], BF16, name="xln")
                    layernorm_chunk(src, t, xln)
                    for fb in range(8):
                        pt = psT1.tile([128, 128], BF16, name="pt")
                        nc.tensor.transpose(pt, xln[:, fb * 128:(fb + 1) * 128],
                                            id_sb)
                        nc.vector.tensor_copy(
                            out=lnT1[:, fb, t * 128:(t + 1) * 128], in_=pt)

                # ---- LN1 + staged AllGather (2 token-halves = batches) ----
                for h in range(2):
                    for tt in range(2):
                        ln_transpose(x_tok, 2 * h + tt)
                    nc.sync.dma_start(
                        out=ag_in[h].rearrange("(p k t) -> p k t", p=128, k=8),
                        in_=lnT1[:, :, h * 256:(h + 1) * 256])
                    nc.gpsimd.collective_compute(
                        "AllGather", mybir.AluOpType.bypass, replica_groups=rg,
                        ins=[ag_in[h].opt()], outs=[ag_out[h].opt()])
                    if h == 0:
                        nc.gpsimd.dma_start(out=bq_sb,
                                            in_=bq.rearrange("b p -> p b"))
                        nc.gpsimd.dma_start(out=bk_sb,
                                            in_=bk.rearrange("b p -> p b"))
                        nc.gpsimd.dma_start(out=me_sb, in_=me)
                        nc.gpsimd.dma_start(out=mo_sb, in_=mo)
                        nc.gpsimd.dma_start(
                            out=bo_sb,
                            in_=bo.rearrange("(k p) -> p k", p=128))
                        nc.gpsimd.dma_start(
                            out=b1_sb,
                            in_=b1.rearrange("(k p) -> p k", p=128))
                        nc.gpsimd.dma_start(
                            out=b2_sb,
                            in_=b2.rearrange("(k p) -> p k", p=128))

                for h in range(2):
                    ago = ag_out[h].rearrange("(r p k t) -> r p k t",
                                              r=NCORES, p=128, k=8)
                    for r in range(NCORES):
                        eng = [nc.sync, nc.scalar][r % 2]
                        eng.dma_start(out=xg[:, r, h], in_=ago[r])

            # ---- attention: kt-major, 2-head row-packed scores ----
            with tc.tile_pool(name="ffp", bufs=1) as ffp:
                # preload the Exp table while ACT is idle (after LN1 Sqrt)
                wrm = ffp.tile([128, 1], F32, name="wrm")
                nc.scalar.activation(out=wrm, in_=eps_sb, func=AF.Exp)
                af_sb = ffp.tile([128, 8, TOK], mybir.dt.float8e4,
                                 name="af_sb")
                wo_sb = ffp.tile([128, 8, D], mybir.dt.float8e4,
                                 name="wo_sb")
                nc.scalar.dma_start(out=wo_sb, in_=wo)
                h1T = ffp.tile([128, 32, TOK], BF16, name="h1T")

                with (
                    tc.tile_pool(name="ptp", bufs=6) as ptp,
                    tc.tile_pool(name="rp", bufs=6) as rp,
                    tc.tile_pool(name="psS", bufs=2, space="PSUM") as psS,
                    tc.tile_pool(name="psO", bufs=3, space="PSUM") as psO,
                ):
                    DR = mybir.MatmulPerfMode.DoubleRow

                    def wo_half(half, accpool, ptpool, acctag, pttag,
                                tbufs=None):
                        # wo projection + residual for token half `half`
                        # (256 tokens); half 0 runs inside batch-1 attention,
                        # hidden under the A2A(b=1) wait.
                        for fb in range(8):
                            accw = accpool.tile([128, 256], F32, name="accw",
                                                tag=acctag, bufs=tbufs)
                            for c in range(4):
                                nc.tensor.matmul(
                                    accw,
                                    lhsT=wo_sb[:, 2 * c:2 * c + 2,
                                               fb * 128:(fb + 1) * 128],
                                    rhs=af_sb[:, 2 * c:2 * c + 2,
                                              half * 256:(half + 1) * 256],
                                    start=(c == 0), stop=(c == 3),
                                    perf_mode=DR)
                            yT = stg.tile([128, 256], BF16, name="yT")
                            nc.vector.tensor_scalar(
                                out=yT, in0=accw, scalar1=1.0 / 64.0,
                                scalar2=bo_sb[:, fb:fb + 1],
                                op0=mybir.AluOpType.mult,
                                op1=mybir.AluOpType.add)
                            for tt in range(2):
                                t = 2 * half + tt
                                ptw = ptpool.tile([128, 128], BF16,
                                                  name="ptw", tag=pttag,
                                                  bufs=tbufs)
                                nc.tensor.transpose(
                                    ptw, yT[:, tt * 128:(tt + 1) * 128],
                                    id_sb)
                                nc.vector.tensor_add(
                                    out=x_tok[:, t, fb * 128:(fb + 1) * 128],
                                    in0=x_tok[:, t, fb * 128:(fb + 1) * 128],
                                    in1=ptw)

                    def ln2_chunk(t, ptpool, pttag, tbufs=None):
                        xln = stg.tile([128, D], BF16, name="xln")
                        layernorm_chunk(x_tok, t, xln)
                        for fb in range(8):
                            ptl = ptpool.tile([128, 128], BF16, name="ptl",
                                              tag=pttag, bufs=tbufs)
                            nc.tensor.transpose(
                                ptl, xln[:, fb * 128:(fb + 1) * 128], id_sb)
                            nc.vector.tensor_copy(
                                out=lnT[:, fb, t * 128:(t + 1) * 128],
                                in_=ptl)

                    def qkv_chunk(h, r):
                        # QKV for tokens [256r, 256(r+1)) of batch h.  PSUM
                        # accumulators share the attention score slots (tag
                        # "sc") so batch-1 chunks can interleave with batch-0
                        # attention rounds without extra PSUM banks.
                        c0 = r * 256
                        for dst, wsb, bsb in ((qT, wq_sb, bq_sb),
                                              (kT, wk_sb, bk_sb)):
                            acc = psS.tile([128, 256], F32, name="acc",
                                           tag="sc")
                            for kc in range(8):
                                nc.tensor.matmul(acc, lhsT=wsb[:, kc, :],
                                                 rhs=xg[:, r, h, kc, :],
                                                 start=(kc == 0),
                                                 stop=(kc == 7))
                            nc.vector.tensor_scalar(
                                out=dst[:, h, c0:c0 + 256], in0=acc,
                                scalar1=1.0 / 64.0,
                                scalar2=bsb,
                                op0=mybir.AluOpType.mult,
                                op1=mybir.AluOpType.add)
                        for tc_ in range(2):
                            accv = psS.tile([128, 128], F32, name="accv",
                                            tag="sc")
                            for kc in range(8):
                                nc.tensor.matmul(
                                    accv,
                                    lhsT=xg[:, r, h, kc,
                                            tc_ * 128:tc_ * 128 + 128],
                                    rhs=wv_sb[:, kc, :],
                                    start=(kc == 0), stop=(kc == 7))
                            ch = h * 16 + 2 * r + tc_
                            nc.vector.tensor_copy(
                                out=vtok[:, ch, :, 0:64],
                                in_=accv.rearrange("p (hd c) -> p hd c",
                                                   hd=2))

                    def emit_pv(b, q2lo_h, q2hi, Ot, m, q2lo_p, pt2):
                        # PV over the kt pair (2m, 2m+1) as one fp8 DoubleRow
                        # matmul (K=256 virtual): lhsT [128,2,128], rhs
                        # [128,2,256] interleaved k-halves.
                        for q2 in range(q2lo_p, q2hi):
                            qr = (q2 - q2lo_p) * 256
                            for hh in range(2):
                                # start=True clears has_written bits for the
                                # WHOLE bank; two q2 share one bank, so only
                                # the bank-first q2 may clear. The sibling's
                                # first write lands on cleared bits and
                                # overwrites correctly.
                                nc.tensor.matmul(
                                    Ot[hh][:, q2 - q2lo_h, :],
                                    lhsT=vtok[:, b * 16 + 2 * m:
                                              b * 16 + 2 * m + 2, hh, :],
                                    rhs=pt2[:, :, hh, qr:qr + 256],
                                    start=(m == 0
                                           and (q2 - q2lo_h) % 2 == 0),
                                    stop=(m == q2),
                                    perf_mode=DR,
                                    skip_group_check=True)

                    # dense QKV for batch 0; batch-1 chunks are spread into
                    # batch-0's attention rounds (PE fills exp-wait gaps)
                    for r in range(NCORES):
                        qkv_chunk(0, r)

                    # quarter-rounds: (b, qq) covers q2 pair (2qq, 2qq+1)
                    for b in range(B):
                        fills = list(range(NCORES)) if b == 0 else []
                        for qq in range(4):
                            q2lo_h, q2hi = 2 * qq, 2 * qq + 2
                            npair = 2 * qq + 2
                            # O (per head): [128, 2 q2, 256]; rows 64:127 get
                            # the denominator replicated via vtok's ones cols
                            Ot = [psO.tile([128, 2, 256], F32, name="O")
                                  for _ in range(2)]
                            pend = []
                            for m in range(npair):
                                q2lo_p = max(q2lo_h, m)
                                ncols = (q2hi - q2lo_p) * 256
                                # probs for kt pair: [p, j, hh, q] fp8
                                pt2 = ptp.tile([128, 2, 2, 512],
                                               mybir.dt.float8e4, name="pt2")
                                for j in range(2):
                                    kt = 2 * m + j
                                    sc = psS.tile([128, 2, 512], F32,
                                                  name="sc")
                                    for hh in range(2):
                                        hp = hh * 64
                                        nc.tensor.matmul(
                                            sc[:, hh, :ncols],
                                            lhsT=kT[hp:hp + 64, b,
                                                    kt * 128:kt * 128 + 128],
                                            rhs=qT[hp:hp + 64, b,
                                                   q2lo_p * 256:
                                                   q2lo_p * 256 + ncols],
                                            start=True, stop=True)
                                    nc.scalar.activation(
                                        out=pt2[:, j, :, :ncols],
                                        in_=sc[:, :, :ncols], func=AF.Exp)
                                    # causal mask on the diagonal 256-block
                                    if m >= q2lo_h:
                                        msk = me_sb if j == 0 else mo_sb
                                        for hh in range(2):
                                            nc.vector.tensor_mul(
                                                out=pt2[:, j, hh, 0:256],
                                                in0=pt2[:, j, hh, 0:256],
                                                in1=msk)
                                # software pipeline, depth 2: PV(m-2) issues
                                # after pair m's score matmuls, so exp(m-2)
                                # has a full pair-period of slack and never
                                # stalls the in-order PE queue
                                pend.append((m, q2lo_p, pt2))
                                if len(pend) > 2:
                                    emit_pv(b, q2lo_h, q2hi, Ot,
                                            *pend.pop(0))
                            for p_ in pend:
                                emit_pv(b, q2lo_h, q2hi, Ot, *p_)

                            # normalize: denominator rows are broadcast by the
                            # matmul; copy to SBUF (approx-recip can't read
                            # PSUM), reciprocal, fused multiply per head
                            for hh in range(2):
                                hp = hh * 64
                                for q2 in range(q2lo_h, q2hi):
                                    O2 = Ot[hh][:, q2 - q2lo_h, :]
                                    den = rp.tile([64, 256], F32, name="den")
                                    nc.vector.tensor_copy(out=den,
                                                          in_=O2[64:128, :])
                                    rc = rp.tile([64, 256], F32, name="rc")
                                    nc.vector.reciprocal_approx_fast(
                                        out=rc, in_=den)
                                    nc.vector.tensor_mul(
                                        out=attnT[hp:hp + 64, b,
                                                  q2 * 256:q2 * 256 + 256],
                                        in0=O2[0:64, :], in1=rc)
                            # fp8-cast this round's block (cast must be a
                            # tensor_copy for neuronxcc); round qq maps
                            # exactly to A2A dest slices {2qq, 2qq+1}, so
                            # stage it immediately
                            nc.vector.tensor_copy(
                                out=attn8[:, b, q2lo_h * 256:q2hi * 256],
                                in_=attnT[:, b, q2lo_h * 256:q2hi * 256])
                            nc.sync.dma_start(
                                out=a2a_in[b].rearrange(
                                    "(s p t) -> p s t", s=8,
                                    p=128)[:, 2 * qq:2 * qq + 2],
                                in_=attn8[:, b, q2lo_h * 256:q2hi * 256]
                                .rearrange("p (s t) -> p s t", s=2))

                            # spread batch-1 QKV into batch-0's rounds: the
                            # PE picks these up while ACT drains this round's
                            # exp backlog
                            for _ in range(2):
                                if fills:
                                    qkv_chunk(1, fills.pop(0))

                        # AllToAll per batch (b=0's collective hides under
                        # the b=1 compute)
                        nc.gpsimd.collective_compute(
                            "AllToAll", mybir.AluOpType.bypass,
                            replica_groups=rg,
                            ins=[a2a_in[b].opt()], outs=[a2a_out[b].opt()])
                        _af3 = a2a_out[b].rearrange("(i p t) -> i p t",
                                                    i=8, p=128)
                        for i in range(8):
                            [nc.gpsimd, nc.scalar][i % 2].dma_start(
                                out=af_sb[:, i, b * 256:(b + 1) * 256],
                                in_=_af3[i])
                        if b == 0:
                            # batch-0's wo + residual + LN2 run inside
                            # batch-1 attention / the A2A(b=1) wait, through
                            # a dedicated 1-bank PSUM slot so they never
                            # gate the attention Ot slots
                            wo_half(0, psO, psO, "wo", "wo", tbufs=1)
                            ln2_chunk(0, psO, "wo", tbufs=1)
                            ln2_chunk(1, psO, "wo", tbufs=1)

                # ---- wo projection + residual; LN2; FFN ----
                with (
                    tc.tile_pool(name="psW", bufs=3, space="PSUM") as psW,
                    tc.tile_pool(name="psT2", bufs=2, space="PSUM") as psT2,
                    tc.tile_pool(name="w2p", bufs=6) as w2p,
                ):
                    def resid_add(ybf, fb):
                        # ybf [128 feat, 512 tok] bf16 -> x_tok += y^T
                        for t in range(4):
                            pt = psT2.tile([128, 128], BF16, name="pt")
                            nc.tensor.transpose(
                                pt, ybf[:, t * 128:(t + 1) * 128], id_sb)
                            nc.vector.tensor_add(
                                out=x_tok[:, t, fb * 128:(fb + 1) * 128],
                                in0=x_tok[:, t, fb * 128:(fb + 1) * 128],
                                in1=pt)

                    # wo + residual + LN2 for the batch-1 token half (the
                    # batch-0 half ran inside the attention region)
                    wo_half(1, psW, psT2, "acc", "pt")
                    ln2_chunk(2, psT2, "pt")
                    ln2_chunk(3, psT2, "pt")

                    # ---- FFN1 ----
                    with tc.tile_pool(name="w1p", bufs=6) as w1p:
                        for hbk in range(32):
                            w1t = w1p.tile([128, 8, 128], BF16, name="w1t")
                            nc.sync.dma_start(out=w1t, in_=w1[:, hbk])
                            acc = psW.tile([128, TOK], F32, name="acc")
                            for kc in range(8):
                                nc.tensor.matmul(acc, lhsT=w1t[:, kc, :],
                                                 rhs=lnT[:, kc, :],
                                                 start=(kc == 0),
                                                 stop=(kc == 7))
                            nc.scalar.activation(out=h1T[:, hbk, :], in_=acc,
                                                 func=AF.Gelu,
                                                 bias=b1_sb[:, hbk:hbk + 1],
                                                 scale=1.0)

                    # ---- FFN2 + residual + streamed output ----
                    for fb in range(8):
                        acc = psW.tile([128, TOK], F32, name="acc")
                        for hg in range(4):
                            w2t = w2p.tile([128, 8, 128], BF16, name="w2t")
                            nc.sync.dma_start(out=w2t, in_=w2[:, fb, hg])
                            for kc in range(8):
                                nc.tensor.matmul(acc, lhsT=w2t[:, kc, :],
                                                 rhs=h1T[:, hg * 8 + kc, :],
                                                 start=(hg == 0 and kc == 0),
                                                 stop=(hg == 3 and kc == 7))
                        y2T = stg.tile([128, TOK], BF16, name="y2T")
                        nc.vector.tensor_scalar_add(out=y2T, in0=acc,
                                                    scalar1=b2_sb[:, fb:fb + 1])
                        resid_add(y2T, fb)
                        ov = out.rearrange("(t p) d -> p t d", p=128)
                        for t in range(4):
                            [nc.sync, nc.scalar][t % 2].dma_start(
                                out=ov[:, t, fb * 128:(fb + 1) * 128],
                                in_=x_tok[:, t, fb * 128:(fb + 1) * 128])

    nc.compile()
    return nc


_NC_CACHE = {}


def _get_nc():
    if "nc" not in _NC_CACHE:
        _NC_CACHE["nc"] = build_nc()
    return _NC_CACHE["nc"]


def _prep_in_maps(x, ln1_g, ln1_b, wq, bq, wk, bk, wv, bv, wo, bo,
                  ln2_g, ln2_b, w1, b1, w2, b2):
    bf16 = ml_dtypes.bfloat16
    f32 = np.float32
    x = np.asarray(x, f32)
    DK = 64
    sc = 1.0 / np.sqrt(DK)
    ln1_g = np.asarray(ln1_g, f32)
    ln1_b = np.asarray(ln1_b, f32)
    ln2_g = np.asarray(ln2_g, f32)
    ln2_b = np.asarray(ln2_b, f32)
    wq = np.asarray(wq, f32)
    wk = np.asarray(wk, f32)
    wv = np.asarray(wv, f32)
    wo_np = np.asarray(wo, f32)
    w1 = np.asarray(w1, f32)
    w2 = np.asarray(w2, f32)

    f8 = ml_dtypes.float8_e4m3
    wq_f = (ln1_g[:, None] * wq * sc * 64.0).astype(f8)
    bq_f = ((ln1_b @ wq + np.asarray(bq, f32)) * sc).astype(f32)
    wk_f = (ln1_g[:, None] * wk * 64.0).astype(f8)
    bk_f = (ln1_b @ wk + np.asarray(bk, f32)).astype(f32)
    wv_f = (ln1_g[:, None] * wv * 64.0).astype(f8)
    bv_f = (ln1_b @ wv + np.asarray(bv, f32)).astype(f32)
    bo_f = (np.asarray(bo, f32) + bv_f @ wo_np).astype(f32)
    wo_f = (wo_np * 64.0).astype(ml_dtypes.float8_e4m3)
    w1_f = (ln2_g[:, None] * w1).astype(bf16)
    b1_f = (ln2_b @ w1 + np.asarray(b1, f32)).astype(f32)
    w2_f = w2.astype(bf16)
    b2_f = np.asarray(b2, f32)

    tri = np.triu(np.ones((128, 128), f32))
    me_np = np.concatenate([tri, np.ones((128, 128), f32)], 1).astype(f8)
    mo_np = np.concatenate([np.zeros((128, 128), f32), tri], 1).astype(f8)
    id_np = np.eye(128, dtype=f32).astype(bf16)

    def pmaj(a):   # [1024, ...] -> [128, 8, ...] partition-major
        return np.ascontiguousarray(
            a.reshape(8, 128, *a.shape[1:]).transpose(
                1, 0, *range(2, a.ndim + 1)))

    w1_pm = np.ascontiguousarray(
        w1_f.reshape(8, 128, 32, 128).transpose(1, 2, 0, 3))
    w2_pm = np.ascontiguousarray(
        w2_f.reshape(4, 8, 128, 8, 128).transpose(2, 3, 0, 1, 4))
    wo_pm = pmaj(wo_f)

    in_maps = []
    for core in range(NCORES):
        # core j: owns tokens [256j, 256(j+1)) of BOTH batches; computes
        # heads {2j, 2j+1} for both batches.
        hs = slice(core * 128, (core + 1) * 128)
        in_maps.append({
            "x_own": np.ascontiguousarray(np.concatenate(
                [x[b, core * 256:(core + 1) * 256, :] for b in range(B)],
                axis=0)),
            "wq": pmaj(wq_f[:, hs]),
            "wk": pmaj(wk_f[:, hs]),
            "wv": pmaj(wv_f[:, hs]),
            "bq": np.ascontiguousarray(bq_f[None, hs]),
            "bk": np.ascontiguousarray(bk_f[None, hs]),
            "wo": wo_pm, "bo": bo_f,
            "w1": w1_pm, "b1": b1_f,
            "w2": w2_pm, "b2": b2_f,
            "me": me_np, "mo": mo_np, "id128": id_np,
        })
    return in_maps


def kernel(**inputs):
    nc = _get_nc()
    in_maps = _prep_in_maps(**inputs)
    res = run_bass_kernel_spmd(nc, in_maps, core_ids=list(range(NCORES)))
    full = np.empty((B, S, D), np.float32)
    for core in range(NCORES):
        o = res.results[core]["out"]
        for b in range(B):
            full[b, core * 256:(core + 1) * 256, :] = \
                o[b * 256:(b + 1) * 256, :]
    return full

